# revision 39
# baseline (speedup 1.0000x reference)
"""ByteBlockTransformerEncoder on 8 NeuronCores (Trainium2, Bass/Tile).

Sharding: data-parallel over batch (B=8 -> one batch element per core),
weights replicated, no collectives. The [S,S] block-diagonal mask is
exploited directly: attention runs per segment (boundaries are known on
host at trace time, so the kernel is specialized to them).

v2 layout notes:
- All matmul operands are bf16 (fp32 matmuls cost 4 cyc/row on PE; bf16 1).
- hT [e, S] bf16 is the working activation; LayerNorm runs in [s, e]
  layout directly from PSUM (bn_stats + fused normalize), and the
  [s,e] -> [e,s] transposes go through single xbar DMA-transpose
  instructions instead of per-tile PE transposes + copies.
- Scores per head are ST[k, q] so the softmax denominator comes from an
  appended ones-column in V during the AV matmul; the denominator
  renormalize is one broadcast tensor_tensor per q-tile.
- q/k live in a padded layout (d=16 rows of a 32-partition tile per
  head) because PE operands must start at partition 0/32/64/96.
- Attention q-tiles are aligned to the global 128 grid so normalized
  outputs can be DMA'd into a contiguous [s, e] buffer and transposed
  to oT with one DMA-transpose.
"""

import sys

import numpy as np

if "/opt/trn_rl_repo" not in sys.path:
    sys.path.insert(0, "/opt/trn_rl_repo")

import os

import concourse.bass as bass
import concourse.tile as tile
from concourse import mybir
from concourse.bass import ts
from concourse.masks import make_identity

B, S, E, H, F, L, V, DH = 8, 2048, 128, 8, 512, 4, 256, 16
NT = S // 128  # 16 s-tiles
NC4 = S // 512  # 4 512-chunks
EPS = 1e-5
SCALE = 1.0 / float(np.sqrt(DH))
f32 = mybir.dt.float32
bf16 = mybir.dt.bfloat16
AF = mybir.ActivationFunctionType
OP = mybir.AluOpType


def _segments(boundaries):
    bs = sorted(int(b) for b in np.asarray(boundaries).reshape(-1))
    edges = [0] + [min(max(b, 0), S) for b in bs] + [S]
    segs = []
    for i in range(len(edges) - 1):
        if edges[i + 1] > edges[i]:
            segs.append((edges[i], edges[i + 1] - edges[i]))
    return segs


def _r32(n):
    return min((n + 31) // 32 * 32, 128)


def _ktiles(start, length):
    out = []
    k = start
    while k < start + length:
        out.append((k, min(128, start + length - k)))
        k += 128
    return out


def _qtiles(start, length):
    """q-tiles aligned to the global 128 grid (each fits one s-tile)."""
    out = []
    q = start
    end = start + length
    while q < end:
        nxt = min((q // 128 + 1) * 128, end)
        out.append((q, nxt - q))
        q = nxt
    return out


def split_multiwaits(nc, drain_max=1, other_max=1):
    """Walrus codegen rejects instructions carrying more sem waits than the
    ISA struct allows (1 for CTRL/Drain, ~2 for compute). Hoist excess waits
    onto same-engine NoOps inserted right before (earlier waits on the same
    engine preserve semantics)."""
    for f in nc.m.functions:
        for bb in f.blocks:
            il = bb.instructions
            i = 0
            while i < len(il):
                ins = il[i]
                si = ins.sync_info
                max_waits = drain_max if type(ins).__name__ == "InstDrain" else other_max
                if si is not None and si.on_wait and len(si.on_wait) > max_waits:
                    waits = list(si.on_wait)
                    si.on_wait = waits[:max_waits]
                    ins.sync_info = si
                    rest = waits[max_waits:]
                    pre = []
                    for j in range(0, len(rest), max_waits):
                        nop = nc.engines[ins.engine].nop(nofuse=True).ins
                        for f2 in nc.m.functions:
                            for bb2 in f2.blocks:
                                il2 = bb2.instructions
                                if il2 and il2[-1].name == nop.name:
                                    il2.pop()
                        nsi = nop.sync_info
                        if nsi is None:
                            nsi = mybir.SyncInfo(
                                on_wait=rest[j : j + max_waits], on_update=[]
                            )
                        else:
                            nsi.on_wait = rest[j : j + max_waits]
                        nop.sync_info = nsi
                        pre.append(nop)
                    il[i:i] = pre
                    i += len(pre)
                i += 1


def build(nc, segs, use_bias, repeat=1):
    x_d = nc.dram_tensor("xf", [1, S], bf16, kind="ExternalInput").ap()
    embt_d = nc.dram_tensor("embt", [128, 2 * E], bf16, kind="ExternalInput").ap()
    iota_d = nc.dram_tensor("iota", [128, 2], f32, kind="ExternalInput").ap()
    wv_d = nc.dram_tensor("wvT", [128, L, E], bf16, kind="ExternalInput").ap()
    # per-head q/k projection weights; [e, L, (q|k), h, d]
    wqkh_d = nc.dram_tensor("wqkh", [128, L, 2, H, DH], bf16, kind="ExternalInput").ap()
    woT_d = nc.dram_tensor("woT", [128, L, E], bf16, kind="ExternalInput").ap()
    w1T_d = nc.dram_tensor("w1T", [128, L, F], bf16, kind="ExternalInput").ap()
    w2T_d = nc.dram_tensor("w2T", [128, L, 4, E], bf16, kind="ExternalInput").ap()
    out_d = nc.dram_tensor("out", [128, 1], f32, kind="ExternalOutput").ap()
    bias_d = {}
    if use_bias["bqkv"]:
        bias_d["bqk_c"] = nc.dram_tensor("bqk_c", [128, L, 2], f32, kind="ExternalInput").ap()
        bias_d["bqkv_v"] = nc.dram_tensor("bqkv_v", [1, L, E], f32, kind="ExternalInput").ap()
    if use_bias["bo"]:
        bias_d["bo_r"] = nc.dram_tensor("bo_r", [1, L, E], f32, kind="ExternalInput").ap()
    if use_bias["b1"]:
        bias_d["b1_c"] = nc.dram_tensor("b1_c", [128, L, 4], f32, kind="ExternalInput").ap()
    if use_bias["b2"]:
        bias_d["b2_r"] = nc.dram_tensor("b2_r", [1, L, E], f32, kind="ExternalInput").ap()
    if use_bias["ln"]:
        bias_d["ln_gb"] = nc.dram_tensor(
            "ln_gb", [1, L, 2, 2, E], f32, kind="ExternalInput"
        ).ap()  # [l, which_ln, g|b, e]

    seg_plans = [(_ktiles(s0, ln), _qtiles(s0, ln)) for s0, ln in segs]
    nkt_max = max(len(kt) for kt, _ in seg_plans)

    with tile.TileContext(nc) as tc:
        with (
            tc.tile_pool(name="const", bufs=1) as constp,
            tc.tile_pool(name="state", bufs=1) as statep,
            tc.tile_pool(name="se", bufs=2) as sep,
            tc.tile_pool(name="tl", bufs=2) as tlp,
            tc.tile_pool(name="p512", bufs=2, space="PSUM") as p512,
            tc.tile_pool(name="st", bufs=2, space="PSUM") as stp,
            tc.tile_pool(name="oap", bufs=2, space="PSUM") as oap,
            tc.tile_pool(name="pt", bufs=2 * nkt_max + 2) as ptp,
            tc.tile_pool(name="small", bufs=8) as smallp,
            tc.tile_pool(name="oseg", bufs=6) as osegp,
            tc.tile_pool(name="ffn", bufs=2) as ffnp,
        ):
            # ---- constants ----
            ident = constp.tile([128, 128], bf16)
            make_identity(nc, ident)
            ident_f = constp.tile([128, 128], f32)
            make_identity(nc, ident_f)
            ones_col = constp.tile([128, 1], bf16)
            nc.vector.memset(ones_col, 1.0)
            eps_col = constp.tile([128, 1], f32)
            nc.vector.memset(eps_col, EPS)
            iota_sb = constp.tile([128, 2], f32)
            nc.sync.dma_start(out=iota_sb, in_=iota_d)
            embt = constp.tile([128, 2, E], bf16)
            nc.sync.dma_start(out=embt, in_=embt_d.rearrange("p (t e) -> p t e", t=2))
            # all-layer weights, loaded once
            wv_all = constp.tile([128, L, E], bf16, name="wv_all")
            nc.sync.dma_start(out=wv_all, in_=wv_d)
            wqkh_all = constp.tile([128, L, 2, H, DH], bf16, name="wqkh_all")
            nc.sync.dma_start(out=wqkh_all, in_=wqkh_d)
            woT_all = constp.tile([128, L, E], bf16, name="woT_all")
            nc.sync.dma_start(out=woT_all, in_=woT_d)
            w1T_all = constp.tile([128, L, F], bf16, name="w1T_all")
            nc.sync.dma_start(out=w1T_all, in_=w1T_d)
            w2T_all = constp.tile([128, L, 4, E], bf16, name="w2T_all")
            nc.sync.dma_start(out=w2T_all, in_=w2T_d)
            bias_sb = {}
            for name, d in bias_d.items():
                if name in ("bqkv_v", "bo_r", "b2_r", "ln_gb"):
                    sh = [128] + list(d.shape[1:])
                    t = constp.tile(sh, f32, name=name)
                    nc.sync.dma_start(out=t, in_=d.to_broadcast(sh))
                else:
                    t = constp.tile(list(d.shape), f32, name=name)
                    nc.sync.dma_start(out=t, in_=d)
                bias_sb[name] = t
            ones_row = None
            if any(use_bias.values()):
                ones_row = constp.tile([1, 128], bf16)
                nc.vector.memset(ones_row, 1.0)


            # persistent per-(segment, ktile) v tiles [k, h, d | ones]
            va_tiles = {}
            for si, (ktiles, _) in enumerate(seg_plans):
                for ki, (ks, kl) in enumerate(ktiles):
                    va = statep.tile(
                        [128, H, 17], bf16, tag=f"va_{si}_{ki}", name=f"va_{si}_{ki}"
                    )
                    nc.vector.memset(va, 0.0)
                    nc.vector.memset(va[:kl, :, 16:17], 1.0)
                    va_tiles[(si, ki)] = va

            # hT double buffer (flat [e, S+32]; pad cols stay zero)
            hT_tiles = [
                statep.tile([128, S + 32], bf16, tag=f"hT{i}", name=f"hT{i}")
                for i in range(2)
            ]
            for t in hT_tiles:
                nc.vector.memset(t[:, S : S + 32], 0.0)

            # ---- embedding via one-hot matmul (chunked), direct to hT ----
            hT = hT_tiles[0]
            with tc.tile_pool(name="emb_tmp", bufs=2) as embp:
                for c in range(NC4):
                    xbc = embp.tile([128, 512], bf16, tag="xbc")
                    nc.sync.dma_start(
                        out=xbc, in_=x_d[:, ts(c, 512)].to_broadcast([128, 512])
                    )
                    oh = embp.tile([128, 2, 512], bf16, tag="oh")
                    for vt in range(2):
                        nc.vector.tensor_scalar(
                            out=oh[:, vt, :],
                            in0=xbc,
                            scalar1=iota_sb[:, vt : vt + 1],
                            scalar2=None,
                            op0=OP.is_equal,
                        )
                    ps = p512.tile([128, 512], f32, tag="mm512")
                    for vt in range(2):
                        nc.tensor.matmul(
                            ps,
                            lhsT=embt[:, vt, :],
                            rhs=oh[:, vt, :],
                            start=(vt == 0),
                            stop=(vt == 1),
                        )
                    nc.scalar.activation(out=hT[:, ts(c, 512)], in_=ps, func=AF.Copy)

            # ---- q/k projection helpers (layer-pipelined) ----
            # scores read q/k per head from qT2/kT2 [32(d, rows 16-31 zero), h, s]
            # (PE contracts the full 32-row group, so the pad rows must be 0;
            # they are zeroed once here, the per-layer DMAs only write rows 0-15)
            qT2 = statep.tile([32, H, S + 32], bf16, tag="qT2")
            kT2 = statep.tile([32, H, S + 32], bf16, tag="kT2")
            nc.gpsimd.memset(qT2, 0.0)
            nc.gpsimd.memset(kT2, 0.0)
            qk_tiles = {}

            def alloc_qk_tiles(li_):
                qk_tmp_ = tlp.tile([128, 2, S + 32], bf16, tag="qk_tmp", name="qk_tmp")
                nc.vector.memset(qk_tmp_[:, :, S : S + 32], 0.0)
                qk_tiles[li_] = qk_tmp_

            def emit_qk_chunk(li_, l_, hT_, c):
                qk_tmp_ = qk_tiles[li_]
                for qk in range(2):
                    ps = p512.tile([128, 512], f32, tag="mm512", name="qkps")
                    nc.tensor.matmul(
                        ps,
                        lhsT=wqkh_all[:, l_, qk, :, :].rearrange("e h d -> e (h d)"),
                        rhs=hT_[:, ts(c, 512)],
                        start=True,
                        stop=not use_bias["bqkv"],
                    )
                    if use_bias["bqkv"]:
                        nc.tensor.matmul(
                            ps,
                            lhsT=ones_row,
                            rhs=bias_sb["bqk_c"][:, l_, qk : qk + 1].to_broadcast(
                                [1, 512]
                            ),
                            start=False,
                            stop=True,
                        )
                    if qk == 0:
                        nc.scalar.activation(
                            out=qk_tmp_[:, qk, ts(c, 512)], in_=ps, func=AF.Copy
                        )
                    else:
                        nc.vector.tensor_copy(
                            out=qk_tmp_[:, qk, ts(c, 512)], in_=ps
                        )
                if c == NC4 - 1:
                    # rearrange heads into the 32-padded base-0 layout (DMA has
                    # no partition-alignment limits); includes the zeroed pad
                    # cols so k-tile overruns read zeros
                    for qk, dst in ((0, qT2), (1, kT2)):
                        for hh in range(H):
                            eng = nc.sync if (qk * H + hh) % 2 else nc.scalar
                            eng.dma_start(
                                out=dst[0:16, hh, :],
                                in_=qk_tmp_[16 * hh : 16 * hh + 16, qk, :],
                            )

            # ---- layers ----
            layers = list(range(L)) * repeat
            h = None
            for li, l in enumerate(layers):
                is_last = li == len(layers) - 1
                hT = hT_tiles[li % 2]
                wv_l = wv_all[:, l, :]
                woT_l = woT_all[:, l, :]
                w1T_l = w1T_all[:, l, :]
                w2T_l = w2T_all[:, l, :, :]

                # q/k tiles for this layer: allocated (and chunks emitted) by
                # the previous layer's tail; layer 0 emits them here.
                if li not in qk_tiles:
                    alloc_qk_tiles(li)
                    for c in range(NC4):
                        emit_qk_chunk(li, l, hT, c)
                qk_tiles.pop(li)

                # v per segment k-tile -> persistent va tiles
                for si, (ktiles, qtiles) in enumerate(seg_plans):
                    for ki, (ks, kl) in enumerate(ktiles):
                        kl32 = _r32(kl)
                        om = oap.tile([128, 512], f32, tag="oa", name=f"v_{si}_{ki}")
                        vps = om[:, 0:E]
                        nc.tensor.matmul(
                            vps[:kl32, :],
                            lhsT=hT[:, ks : ks + kl32],
                            rhs=wv_l,
                            start=True,
                            stop=not use_bias["bqkv"],
                        )
                        if use_bias["bqkv"]:
                            nc.tensor.matmul(
                                vps[:kl32, :],
                                lhsT=ones_row[:, :kl32],
                                rhs=bias_sb["bqkv_v"][0:1, l, :],
                                start=False,
                                stop=True,
                            )
                        nc.vector.tensor_copy(
                            out=va_tiles[(si, ki)][:kl, :, 0:16],
                            in_=vps[:kl, :].rearrange("k (h d) -> k h d", h=H),
                        )

                # attention q-tile stream with the layer tail (z/LN1/FFN/LN2)
                # emitted inline per 512-chunk as soon as its oT columns are
                # complete, so the tail overlaps the rest of attention.
                oT = tlp.tile([128, S + 32], bf16, tag="oT")
                h1 = sep.tile([128, NT, E], bf16, tag="h1")
                h1T = tlp.tile([128, NT, 128], bf16, tag="h1T")
                h = sep.tile([128, NT, E], bf16, tag="h")

                def tail_z(c):
                    zc = oap.tile([128, 512], f32, tag="oa", name=f"z_{li}_{c}")
                    stats = smallp.tile([128, 4, 6], f32, tag="stats", name="st1")
                    mv = smallp.tile([128, 4, 2], f32, tag="mv", name="mv1")
                    rstd = smallp.tile([128, 4], f32, tag="rstd", name="rstd1")
                    for tt in range(4):
                        t = 4 * c + tt
                        zt = zc[:, ts(tt, 128)]
                        nc.tensor.matmul(
                            zt, lhsT=hT[:, ts(t, 128)], rhs=ident, start=True, stop=False
                        )
                        nc.tensor.matmul(
                            zt,
                            lhsT=oT[:, ts(t, 128)],
                            rhs=woT_l,
                            start=False,
                            stop=not use_bias["bo"],
                        )
                        if use_bias["bo"]:
                            nc.tensor.matmul(
                                zt,
                                lhsT=ones_row,
                                rhs=bias_sb["bo_r"][0:1, l, :],
                                start=False,
                                stop=True,
                            )
                        nc.vector.bn_stats(out=stats[:, tt, :], in_=zt)
                        nc.vector.bn_aggr(out=mv[:, tt, :], in_=stats[:, tt, :])
                    nc.scalar.activation(
                        out=rstd, in_=mv[:, :, 1], func=AF.Sqrt, bias=eps_col
                    )
                    nc.vector.reciprocal(out=rstd, in_=rstd)
                    tpc = p512.tile([128, 512], f32, tag="mm512", name="tpc")
                    tpb = tpc.bitcast(bf16)
                    for tt in range(4):
                        t = 4 * c + tt
                        nc.vector.tensor_scalar(
                            out=h1[:, t, :],
                            in0=zc[:, ts(tt, 128)],
                            scalar1=mv[:, tt, 0:1],
                            scalar2=rstd[:, tt : tt + 1],
                            op0=OP.subtract,
                            op1=OP.mult,
                        )
                        if use_bias["ln"]:
                            nc.vector.tensor_mul(
                                out=h1[:, t, :], in0=h1[:, t, :],
                                in1=bias_sb["ln_gb"][:, l, 0, 0, :],
                            )
                            nc.vector.tensor_add(
                                out=h1[:, t, :], in0=h1[:, t, :],
                                in1=bias_sb["ln_gb"][:, l, 0, 1, :],
                            )
                        nc.tensor.transpose(
                            tpb[:, ts(tt, 128)], h1[:, t, :], ident
                        )
                    nc.vector.tensor_copy(
                        out=h1T[:, 4 * c : 4 * c + 4, :], in_=tpb[:, 0:512]
                    )

                def tail_ffn(c):
                    fTc = ffnp.tile([128, 4, 512], bf16, tag="fTc", name="fTc")
                    yc = oap.tile([128, 512], f32, tag="oa", name="yc")
                    stats2 = smallp.tile([128, 4, 6], f32, tag="stats", name="st2")
                    mv2 = smallp.tile([128, 4, 2], f32, tag="mv", name="mv2")
                    rstd2 = smallp.tile([128, 4], f32, tag="rstd", name="rstd2")
                    for tt in range(4):
                        t = 4 * c + tt
                        fps = p512.tile([128, 4, 128], f32, tag="mm512", name="fps")
                        for jt in range(4):
                            nc.tensor.matmul(
                                fps[:, jt, :],
                                lhsT=w1T_l[:, ts(jt, 128)],
                                rhs=h1T[:, t, :],
                                start=True,
                                stop=not use_bias["b1"],
                            )
                            if use_bias["b1"]:
                                nc.tensor.matmul(
                                    fps[:, jt, :],
                                    lhsT=ones_row,
                                    rhs=bias_sb["b1_c"][:, l, jt : jt + 1]
                                    .to_broadcast([1, 128]),
                                    start=False,
                                    stop=True,
                                )
                        if tt % 2:
                            nc.scalar.activation(
                                out=fTc[:, :, ts(tt, 128)], in_=fps, func=AF.Relu
                            )
                        else:
                            nc.vector.tensor_scalar(
                                out=fTc[:, :, ts(tt, 128)],
                                in0=fps,
                                scalar1=0.0,
                                scalar2=None,
                                op0=OP.max,
                            )
                        yt = yc[:, ts(tt, 128)]
                        nc.tensor.matmul(
                            yt, lhsT=h1T[:, t, :], rhs=ident, start=True, stop=False
                        )
                        for ft in range(4):
                            nc.tensor.matmul(
                                yt,
                                lhsT=fTc[:, ft, ts(tt, 128)],
                                rhs=w2T_l[:, ft, :],
                                start=False,
                                stop=(ft == 3) and not use_bias["b2"],
                            )
                        if use_bias["b2"]:
                            nc.tensor.matmul(
                                yt,
                                lhsT=ones_row,
                                rhs=bias_sb["b2_r"][0:1, l, :],
                                start=False,
                                stop=True,
                            )
                        nc.vector.bn_stats(out=stats2[:, tt, :], in_=yt)
                        nc.vector.bn_aggr(out=mv2[:, tt, :], in_=stats2[:, tt, :])
                    nc.scalar.activation(
                        out=rstd2, in_=mv2[:, :, 1], func=AF.Sqrt, bias=eps_col
                    )
                    nc.vector.reciprocal(out=rstd2, in_=rstd2)
                    if not is_last:
                        tpc2 = p512.tile([128, 512], f32, tag="mm512", name="tpc2")
                        tpb2 = tpc2.bitcast(bf16)
                    for tt in range(4):
                        t = 4 * c + tt
                        nc.vector.tensor_scalar(
                            out=h[:, t, :],
                            in0=yc[:, ts(tt, 128)],
                            scalar1=mv2[:, tt, 0:1],
                            scalar2=rstd2[:, tt : tt + 1],
                            op0=OP.subtract,
                            op1=OP.mult,
                        )
                        if use_bias["ln"]:
                            nc.vector.tensor_mul(
                                out=h[:, t, :], in0=h[:, t, :],
                                in1=bias_sb["ln_gb"][:, l, 1, 0, :],
                            )
                            nc.vector.tensor_add(
                                out=h[:, t, :], in0=h[:, t, :],
                                in1=bias_sb["ln_gb"][:, l, 1, 1, :],
                            )
                        if not is_last:
                            nc.tensor.transpose(
                                tpb2[:, ts(tt, 128)], h[:, t, :], ident
                            )
                    if not is_last:
                        hT_next = hT_tiles[(li + 1) % 2]
                        nc.vector.tensor_copy(
                            out=hT_next[:, ts(c, 512)], in_=tpb2[:, 0:512]
                        )
                        if (li + 1) not in qk_tiles:
                            alloc_qk_tiles(li + 1)
                        emit_qk_chunk(li + 1, layers[li + 1], hT_next, c)

                # grid tiles: all segment pieces within one 128-wide q tile
                # share st/pt (scores land at their q offsets in the free dim)
                # and one exp per k-depth, plus one oT copy per tile.
                grid = []
                for g in range(NT):
                    lo, hi = g * 128, (g + 1) * 128
                    pieces = []
                    for si, (ktiles, qtiles) in enumerate(seg_plans):
                        for qs, qn in qtiles:
                            if lo <= qs < hi:
                                pieces.append((si, ktiles, qs, qn))
                    grid.append(pieces)

                def emit_scores_tile(g):
                    pieces = grid[g]
                    mk = max(len(kt) for _, kt, _, _ in pieces)
                    pts = []
                    for ki in range(mk):
                        pk = [p for p in pieces if len(p[1]) > ki]
                        xlo = min(qs for _, _, qs, _ in pk) - 128 * g
                        xhi = max(qs + qn for _, _, qs, qn in pk) - 128 * g
                        st = stp.tile([128, H, 128], f32, tag="st", name="st")
                        for si, kt, qs, qn in pk:
                            ks, kl = kt[ki]
                            kl32 = _r32(kl)
                            qo = qs - 128 * g
                            for hh in range(H):
                                nc.tensor.matmul(
                                    st[:kl32, hh, qo : qo + qn],
                                    lhsT=kT2[:, hh, ks : ks + kl32],
                                    rhs=qT2[:, hh, qs : qs + qn],
                                    start=True,
                                    stop=True,
                                )
                        pt = ptp.tile([128, H, 128], bf16, tag="pt", name="pt")
                        nc.scalar.activation(
                            out=pt[:, :, xlo:xhi],
                            in_=st[:, :, xlo:xhi],
                            func=AF.Exp,
                            scale=SCALE,
                        )
                        pts.append(pt)
                    return pts

                def emit_av_tile(g, pts):
                    # pack up to 3 pieces' AV outputs into one PSUM slot
                    # (piece j at cols j*170) so one divide renormalizes all
                    pieces = grid[g]
                    out = []
                    for base in range(0, len(pieces), 3):
                        grp = pieces[base : base + 3]
                        om = oap.tile([128, 512], f32, tag="oa", name="oa")
                        qmax = 0
                        for j, (si, kt, qs, qn) in enumerate(grp):
                            nkt = len(kt)
                            qo = qs - 128 * g
                            qmax = max(qmax, qn)
                            oa = om[:, 170 * j : 170 * j + 136].rearrange(
                                "p (h x) -> p h x", h=H
                            )
                            for hh in range(H):
                                for ki in range(nkt):
                                    kl32 = _r32(kt[ki][1])
                                    nc.tensor.matmul(
                                        oa[:qn, hh, 0:17],
                                        lhsT=pts[ki][:kl32, hh, qo : qo + qn],
                                        rhs=va_tiles[(si, ki)][:kl32, hh, :],
                                        start=(ki == 0),
                                        stop=(ki == nkt - 1),
                                    )
                        oag = om[:, 0:510].rearrange(
                            "p (j x) -> p j x", j=3
                        )[:, 0 : len(grp), 0:136].rearrange(
                            "p j (h x) -> p j h x", h=H
                        )
                        rec = smallp.tile([128, 3, H], f32, tag="rec", name="rec")
                        nc.vector.reciprocal(
                            out=rec[:qmax, 0 : len(grp), :],
                            in_=oag[:qmax, :, :, 16],
                        )
                        oseg = osegp.tile([128, 3, E], f32, tag="oseg", name="oseg")
                        nc.vector.tensor_tensor(
                            out=oseg[:qmax, 0 : len(grp), :].rearrange(
                                "q j (h d) -> q j h d", h=H
                            ),
                            in0=oag[:qmax, :, :, 0:16],
                            in1=rec[:qmax, 0 : len(grp), :, None].to_broadcast(
                                [qmax, len(grp), H, 16]
                            ),
                            op=OP.mult,
                        )
                        out.append((om, oseg, grp))
                    return out

                def emit_transpose_tile(g, avs):
                    om0 = avs[0][0]
                    tp = om0[:, 144:272]
                    for om, oseg, grp in avs:
                        for j, (si, kt, qs, qn) in enumerate(grp):
                            qo = qs - 128 * g
                            nc.tensor.transpose(
                                tp[:, qo : qo + qn],
                                oseg[:qn, j, :],
                                ident_f[:qn, :qn],
                            )
                    nc.vector.tensor_copy(
                        out=oT[:, 128 * g : 128 * (g + 1)], in_=tp
                    )

                # 3-stage software pipeline per grid tile g:
                #   scores/exp(g+1) | transpose/copy(g-1) | AV+renorm(g)
                # plus chunk tails staggered in, so no engine waits in-order
                # on a cross-engine producer that hasn't been given slack.
                if os.environ.get("KDBG") == "noattn":
                    nc.vector.memset(oT[:, 0:S], 0.0)
                    for c in range(NC4):
                        tail_z(c)
                        tail_ffn(c)
                    continue
                sprev = None
                aprev = None
                todo = []
                for g in range(NT):
                    pts = emit_scores_tile(g)
                    if todo:
                        todo.pop(0)()
                    if aprev is not None:
                        emit_transpose_tile(aprev[0], aprev[1])
                        if aprev[0] % 4 == 3:
                            c = aprev[0] // 4
                            todo.append(lambda c=c: tail_z(c))
                            todo.append(lambda c=c: tail_ffn(c))
                    if sprev is not None:
                        avs = emit_av_tile(sprev[0], sprev[1])
                        aprev = (sprev[0], avs)
                    sprev = (g, pts)
                avs = emit_av_tile(sprev[0], sprev[1])
                emit_transpose_tile(aprev[0], aprev[1])
                emit_transpose_tile(sprev[0], avs)
                for f in todo:
                    f()
                tail_z(3)
                tail_ffn(3)

            # ---- mean pool over s ----
            om = oap.tile([128, H, 18], f32, tag="oa")
            acc = om[:, 0, 0:1]
            for t in range(NT):
                nc.tensor.matmul(
                    acc,
                    lhsT=h[:, t, :],
                    rhs=ones_col,
                    start=(t == 0),
                    stop=(t == NT - 1),
                )
            out_sb = smallp.tile([128, 1], f32, tag="out")
            nc.scalar.mul(out=out_sb, in_=acc, mul=1.0 / S)
            nc.sync.dma_start(out=out_d, in_=out_sb)

    split_multiwaits(nc)
    return nc


def _to_bf16(a):
    import ml_dtypes

    return np.asarray(a, np.float32).astype(ml_dtypes.bfloat16)


def _prep(x, boundaries, emb, Wqkv, bqkv, Wo, bo, W1, b1, W2, b2,
          ln1_g, ln1_b, ln2_g, ln2_b):
    x = np.asarray(x)
    emb = np.asarray(emb, np.float32)
    Wqkv = np.asarray(Wqkv, np.float32)
    Wo = np.asarray(Wo, np.float32)
    W1 = np.asarray(W1, np.float32)
    W2 = np.asarray(W2, np.float32)
    bqkv = np.asarray(bqkv, np.float32)
    bo = np.asarray(bo, np.float32)
    b1 = np.asarray(b1, np.float32)
    b2 = np.asarray(b2, np.float32)
    ln1_g = np.asarray(ln1_g, np.float32)
    ln1_b = np.asarray(ln1_b, np.float32)
    ln2_g = np.asarray(ln2_g, np.float32)
    ln2_b = np.asarray(ln2_b, np.float32)

    segs = _segments(boundaries)
    use_bias = {
        "bqkv": bool(np.any(bqkv != 0)),
        "bo": bool(np.any(bo != 0)),
        "b1": bool(np.any(b1 != 0)),
        "b2": bool(np.any(b2 != 0)),
        "ln": bool(
            np.any(ln1_g != 1) or np.any(ln1_b != 0)
            or np.any(ln2_g != 1) or np.any(ln2_b != 0)
        ),
    }

    # per-head q/k projection weights [e, L, qk, h, d]
    wqkh = (
        Wqkv[:, : 2 * E, :].reshape(L, 2, H, DH, E).transpose(4, 0, 1, 2, 3).copy()
    )

    shared = {
        "embt": _to_bf16(
            emb.reshape(2, 128, E).transpose(1, 0, 2).reshape(128, 2 * E)
        ),
        "iota": np.arange(V, dtype=np.float32).reshape(2, 128).T.copy(),
        "wvT": _to_bf16(Wqkv[:, 2 * E : 3 * E, :].transpose(2, 0, 1)),  # [e, L, E]
        "wqkh": _to_bf16(wqkh),
        "woT": _to_bf16(Wo.transpose(2, 0, 1)),  # [e, L, E]
        "w1T": _to_bf16(W1.transpose(2, 0, 1)),  # [e, L, F]
        "w2T": _to_bf16(
            W2.transpose(0, 2, 1).reshape(L, 4, 128, E).transpose(2, 0, 1, 3)
        ),
    }
    if use_bias["bqkv"]:
        shared["bqk_c"] = bqkv[:, : 2 * E].reshape(L, 2, 128).transpose(2, 0, 1).copy()
        shared["bqkv_v"] = bqkv[:, 2 * E : 3 * E].reshape(1, L, E).copy()
    if use_bias["bo"]:
        shared["bo_r"] = bo.reshape(1, L, E).copy()
    if use_bias["b1"]:
        shared["b1_c"] = b1.reshape(L, 4, 128).transpose(2, 0, 1).copy()
    if use_bias["b2"]:
        shared["b2_r"] = b2.reshape(1, L, E).copy()
    if use_bias["ln"]:
        ln_gb = np.stack(
            [np.stack([ln1_g, ln1_b], 1), np.stack([ln2_g, ln2_b], 1)], 1
        )  # [L, 2, 2, E]
        shared["ln_gb"] = ln_gb.reshape(1, L, 2, 2, E).copy()

    xf = [_to_bf16(x[b].reshape(1, S)) for b in range(B)]
    return segs, use_bias, shared, xf


def build_from_inputs(repeat=1, **inputs):
    segs, use_bias, shared, xf = _prep(**inputs)
    nc = bass.Bass()
    build(nc, segs, use_bias, repeat=repeat)
    in_maps = [dict(shared, xf=xf[b]) for b in range(B)]
    return nc, in_maps


def kernel(**inputs):
    from concourse.bass_utils import run_bass_kernel_spmd

    nc, in_maps = build_from_inputs(**inputs)
    res = run_bass_kernel_spmd(nc, in_maps, core_ids=list(range(B)))
    out = np.stack([res.results[b]["out"].reshape(E) for b in range(B)])
    return out.astype(np.float32)


# revision 45
# speedup vs baseline: 1120.9849x; 1120.9849x over previous
"""ByteBlockTransformerEncoder on 8 NeuronCores (Trainium2, Bass/Tile).

Sharding: data-parallel over batch (B=8 -> one batch element per core),
weights replicated, no collectives. The [S,S] block-diagonal mask is
exploited directly: attention runs per segment (boundaries are known on
host at trace time, so the kernel is specialized to them).

v2 layout notes:
- All matmul operands are bf16 (fp32 matmuls cost 4 cyc/row on PE; bf16 1).
- hT [e, S] bf16 is the working activation; LayerNorm runs in [s, e]
  layout directly from PSUM (bn_stats + fused normalize), and the
  [s,e] -> [e,s] transposes go through single xbar DMA-transpose
  instructions instead of per-tile PE transposes + copies.
- Scores per head are ST[k, q] so the softmax denominator comes from an
  appended ones-column in V during the AV matmul; the denominator
  renormalize is one broadcast tensor_tensor per q-tile.
- q/k live in a padded layout (d=16 rows of a 32-partition tile per
  head) because PE operands must start at partition 0/32/64/96.
- Attention q-tiles are aligned to the global 128 grid so normalized
  outputs can be DMA'd into a contiguous [s, e] buffer and transposed
  to oT with one DMA-transpose.
"""

import sys

import numpy as np

if "/opt/trn_rl_repo" not in sys.path:
    sys.path.insert(0, "/opt/trn_rl_repo")

import os

import concourse.bass as bass
import concourse.tile as tile
from concourse import mybir
from concourse.bass import ts
from concourse.masks import make_identity

B, S, E, H, F, L, V, DH = 8, 2048, 128, 8, 512, 4, 256, 16
NT = S // 128  # 16 s-tiles
NC4 = S // 512  # 4 512-chunks
EPS = 1e-5
SCALE = 1.0 / float(np.sqrt(DH))
f32 = mybir.dt.float32
bf16 = mybir.dt.bfloat16
AF = mybir.ActivationFunctionType
OP = mybir.AluOpType


def _segments(boundaries):
    bs = sorted(int(b) for b in np.asarray(boundaries).reshape(-1))
    edges = [0] + [min(max(b, 0), S) for b in bs] + [S]
    segs = []
    for i in range(len(edges) - 1):
        if edges[i + 1] > edges[i]:
            segs.append((edges[i], edges[i + 1] - edges[i]))
    return segs


def _r32(n):
    return min((n + 31) // 32 * 32, 128)


def _ktiles(start, length):
    out = []
    k = start
    while k < start + length:
        out.append((k, min(128, start + length - k)))
        k += 128
    return out


def _qtiles(start, length):
    """q-tiles aligned to the global 128 grid (each fits one s-tile)."""
    out = []
    q = start
    end = start + length
    while q < end:
        nxt = min((q // 128 + 1) * 128, end)
        out.append((q, nxt - q))
        q = nxt
    return out


def split_multiwaits(nc, drain_max=1, other_max=1):
    """Walrus codegen rejects instructions carrying more sem waits than the
    ISA struct allows (1 for CTRL/Drain, ~2 for compute). Hoist excess waits
    onto same-engine NoOps inserted right before (earlier waits on the same
    engine preserve semantics)."""
    for f in nc.m.functions:
        for bb in f.blocks:
            il = bb.instructions
            i = 0
            while i < len(il):
                ins = il[i]
                si = ins.sync_info
                max_waits = drain_max if type(ins).__name__ == "InstDrain" else other_max
                if si is not None and si.on_wait and len(si.on_wait) > max_waits:
                    waits = list(si.on_wait)
                    si.on_wait = waits[:max_waits]
                    ins.sync_info = si
                    rest = waits[max_waits:]
                    pre = []
                    for j in range(0, len(rest), max_waits):
                        nop = nc.engines[ins.engine].nop(nofuse=True).ins
                        for f2 in nc.m.functions:
                            for bb2 in f2.blocks:
                                il2 = bb2.instructions
                                if il2 and il2[-1].name == nop.name:
                                    il2.pop()
                        nsi = nop.sync_info
                        if nsi is None:
                            nsi = mybir.SyncInfo(
                                on_wait=rest[j : j + max_waits], on_update=[]
                            )
                        else:
                            nsi.on_wait = rest[j : j + max_waits]
                        nop.sync_info = nsi
                        pre.append(nop)
                    il[i:i] = pre
                    i += len(pre)
                i += 1


def build(nc, segs, use_bias, repeat=1):
    x_d = nc.dram_tensor("xf", [1, S], bf16, kind="ExternalInput").ap()
    embt_d = nc.dram_tensor("embt", [128, 2 * E], bf16, kind="ExternalInput").ap()
    iota_d = nc.dram_tensor("iota", [128, 2], f32, kind="ExternalInput").ap()
    wv_d = nc.dram_tensor("wvT", [128, L, E], bf16, kind="ExternalInput").ap()
    # per-head q/k projection weights; [e, L, (q|k), h, d]
    wqkh_d = nc.dram_tensor("wqkh", [128, L, 2, H, DH], bf16, kind="ExternalInput").ap()
    woT_d = nc.dram_tensor("woT", [128, L, E], bf16, kind="ExternalInput").ap()
    w1T_d = nc.dram_tensor("w1T", [128, L, F], bf16, kind="ExternalInput").ap()
    w2T_d = nc.dram_tensor("w2T", [128, L, 4, E], bf16, kind="ExternalInput").ap()
    out_d = nc.dram_tensor("out", [128, 1], f32, kind="ExternalOutput").ap()
    bias_d = {}
    if use_bias["bqkv"]:
        bias_d["bqk_c"] = nc.dram_tensor("bqk_c", [128, L, 2], f32, kind="ExternalInput").ap()
        bias_d["bqkv_v"] = nc.dram_tensor("bqkv_v", [1, L, E], f32, kind="ExternalInput").ap()
    if use_bias["bo"]:
        bias_d["bo_r"] = nc.dram_tensor("bo_r", [1, L, E], f32, kind="ExternalInput").ap()
    if use_bias["b1"]:
        bias_d["b1_c"] = nc.dram_tensor("b1_c", [128, L, 4], f32, kind="ExternalInput").ap()
    if use_bias["b2"]:
        bias_d["b2_r"] = nc.dram_tensor("b2_r", [1, L, E], f32, kind="ExternalInput").ap()
    if use_bias["ln"]:
        bias_d["ln_gb"] = nc.dram_tensor(
            "ln_gb", [1, L, 2, 2, E], f32, kind="ExternalInput"
        ).ap()  # [l, which_ln, g|b, e]

    seg_plans = [(_ktiles(s0, ln), _qtiles(s0, ln)) for s0, ln in segs]
    nkt_max = max(len(kt) for kt, _ in seg_plans)

    with tile.TileContext(nc) as tc:
        with (
            tc.tile_pool(name="const", bufs=1) as constp,
            tc.tile_pool(name="state", bufs=1) as statep,
            tc.tile_pool(name="se", bufs=2) as sep,
            tc.tile_pool(name="tl", bufs=2) as tlp,
            tc.tile_pool(name="p512", bufs=2, space="PSUM") as p512,
            tc.tile_pool(name="st", bufs=2, space="PSUM") as stp,
            tc.tile_pool(name="oap", bufs=2, space="PSUM") as oap,
            tc.tile_pool(name="pt", bufs=2 * nkt_max + 2) as ptp,
            tc.tile_pool(name="small", bufs=8) as smallp,
            tc.tile_pool(name="oseg", bufs=6) as osegp,
            tc.tile_pool(name="ffn", bufs=2) as ffnp,
        ):
            # ---- constants ----
            ident = constp.tile([128, 128], bf16)
            make_identity(nc, ident)
            ident_f = constp.tile([128, 128], f32)
            make_identity(nc, ident_f)
            ones_col = constp.tile([128, 1], bf16)
            nc.vector.memset(ones_col, 1.0)
            eps_col = constp.tile([128, 1], f32)
            nc.vector.memset(eps_col, EPS)
            iota_sb = constp.tile([128, 2], f32)
            nc.sync.dma_start(out=iota_sb, in_=iota_d)
            embt = constp.tile([128, 2, E], bf16)
            nc.sync.dma_start(out=embt, in_=embt_d.rearrange("p (t e) -> p t e", t=2))
            # all-layer weights, loaded once
            wv_all = constp.tile([128, L, E], bf16, name="wv_all")
            nc.sync.dma_start(out=wv_all, in_=wv_d)
            wqkh_all = constp.tile([128, L, 2, H, DH], bf16, name="wqkh_all")
            nc.sync.dma_start(out=wqkh_all, in_=wqkh_d)
            woT_all = constp.tile([128, L, E], bf16, name="woT_all")
            nc.sync.dma_start(out=woT_all, in_=woT_d)
            w1T_all = constp.tile([128, L, F], bf16, name="w1T_all")
            nc.sync.dma_start(out=w1T_all, in_=w1T_d)
            w2T_all = constp.tile([128, L, 4, E], bf16, name="w2T_all")
            nc.sync.dma_start(out=w2T_all, in_=w2T_d)
            bias_sb = {}
            for name, d in bias_d.items():
                if name in ("bqkv_v", "bo_r", "b2_r", "ln_gb"):
                    sh = [128] + list(d.shape[1:])
                    t = constp.tile(sh, f32, name=name)
                    nc.sync.dma_start(out=t, in_=d.to_broadcast(sh))
                else:
                    t = constp.tile(list(d.shape), f32, name=name)
                    nc.sync.dma_start(out=t, in_=d)
                bias_sb[name] = t
            ones_row = None
            if any(use_bias.values()):
                ones_row = constp.tile([1, 128], bf16)
                nc.vector.memset(ones_row, 1.0)


            # persistent per-(segment, ktile) v tiles [k, h, d | ones]
            va_tiles = {}
            for si, (ktiles, _) in enumerate(seg_plans):
                for ki, (ks, kl) in enumerate(ktiles):
                    va = statep.tile(
                        [128, H, 17], bf16, tag=f"va_{si}_{ki}", name=f"va_{si}_{ki}"
                    )
                    nc.vector.memset(va, 0.0)
                    nc.vector.memset(va[:kl, :, 16:17], 1.0)
                    va_tiles[(si, ki)] = va

            # hT double buffer (flat [e, S+32]; pad cols stay zero)
            hT_tiles = [
                statep.tile([128, S + 32], bf16, tag=f"hT{i}", name=f"hT{i}")
                for i in range(2)
            ]
            for t in hT_tiles:
                nc.vector.memset(t[:, S : S + 32], 0.0)

            # ---- embedding via one-hot matmul (chunked), direct to hT ----
            hT = hT_tiles[0]
            with tc.tile_pool(name="emb_tmp", bufs=2) as embp:
                for c in range(NC4):
                    xbc = embp.tile([128, 512], bf16, tag="xbc")
                    nc.sync.dma_start(
                        out=xbc, in_=x_d[:, ts(c, 512)].to_broadcast([128, 512])
                    )
                    oh = embp.tile([128, 2, 512], bf16, tag="oh")
                    for vt in range(2):
                        nc.vector.tensor_scalar(
                            out=oh[:, vt, :],
                            in0=xbc,
                            scalar1=iota_sb[:, vt : vt + 1],
                            scalar2=None,
                            op0=OP.is_equal,
                        )
                    ps = p512.tile([128, 512], f32, tag="mm512")
                    for vt in range(2):
                        nc.tensor.matmul(
                            ps,
                            lhsT=embt[:, vt, :],
                            rhs=oh[:, vt, :],
                            start=(vt == 0),
                            stop=(vt == 1),
                        )
                    nc.scalar.activation(out=hT[:, ts(c, 512)], in_=ps, func=AF.Copy)

            # ---- q/k projection helpers (layer-pipelined) ----
            # scores read q/k per head from qT2/kT2 [32(d, rows 16-31 zero), h, s]
            # (PE contracts the full 32-row group, so the pad rows must be 0;
            # they are zeroed once here, the per-layer DMAs only write rows 0-15)
            qT2 = statep.tile([32, H, S + 32], bf16, tag="qT2")
            kT2 = statep.tile([32, H, S + 32], bf16, tag="kT2")
            nc.gpsimd.memset(qT2, 0.0)
            nc.gpsimd.memset(kT2, 0.0)
            qk_tiles = {}

            def alloc_qk_tiles(li_):
                qk_tmp_ = tlp.tile([128, 2, S + 32], bf16, tag="qk_tmp", name="qk_tmp")
                nc.vector.memset(qk_tmp_[:, :, S : S + 32], 0.0)
                qk_tiles[li_] = qk_tmp_

            def emit_qk_chunk(li_, l_, hT_, c):
                qk_tmp_ = qk_tiles[li_]
                for qk in range(2):
                    ps = p512.tile([128, 512], f32, tag="mm512", name="qkps")
                    nc.tensor.matmul(
                        ps,
                        lhsT=wqkh_all[:, l_, qk, :, :].rearrange("e h d -> e (h d)"),
                        rhs=hT_[:, ts(c, 512)],
                        start=True,
                        stop=not use_bias["bqkv"],
                    )
                    if use_bias["bqkv"]:
                        nc.tensor.matmul(
                            ps,
                            lhsT=ones_row,
                            rhs=bias_sb["bqk_c"][:, l_, qk : qk + 1].to_broadcast(
                                [1, 512]
                            ),
                            start=False,
                            stop=True,
                        )
                    if qk == 0:
                        nc.scalar.activation(
                            out=qk_tmp_[:, qk, ts(c, 512)], in_=ps, func=AF.Copy
                        )
                    else:
                        nc.vector.tensor_copy(
                            out=qk_tmp_[:, qk, ts(c, 512)], in_=ps
                        )
                if c in (1, NC4 - 1):
                    # rearrange heads into the 32-padded base-0 layout (DMA has
                    # no partition-alignment limits), in two half-S batches so
                    # next layer's early segments can start sooner; the second
                    # batch includes the zeroed pad cols for k-tile overruns
                    lo = 0 if c == 1 else 1024
                    hi = 1024 if c == 1 else S + 32
                    for qk, dst in ((0, qT2), (1, kT2)):
                        for hh in range(H):
                            eng = nc.sync if (qk * H + hh) % 2 else nc.scalar
                            eng.dma_start(
                                out=dst[0:16, hh, lo:hi],
                                in_=qk_tmp_[16 * hh : 16 * hh + 16, qk, lo:hi],
                            )

            # ---- layers ----
            layers = list(range(L)) * repeat
            h = None
            for li, l in enumerate(layers):
                is_last = li == len(layers) - 1
                hT = hT_tiles[li % 2]
                wv_l = wv_all[:, l, :]
                woT_l = woT_all[:, l, :]
                w1T_l = w1T_all[:, l, :]
                w2T_l = w2T_all[:, l, :, :]

                # q/k tiles for this layer: allocated (and chunks emitted) by
                # the previous layer's tail; layer 0 emits them here.
                if li not in qk_tiles:
                    alloc_qk_tiles(li)
                    for c in range(NC4):
                        emit_qk_chunk(li, l, hT, c)
                qk_tiles.pop(li)

                # v per segment k-tile -> persistent va tiles
                for si, (ktiles, qtiles) in enumerate(seg_plans):
                    for ki, (ks, kl) in enumerate(ktiles):
                        kl32 = _r32(kl)
                        om = oap.tile([128, 512], f32, tag="oa", name=f"v_{si}_{ki}")
                        vps = om[:, 0:E]
                        nc.tensor.matmul(
                            vps[:kl32, :],
                            lhsT=hT[:, ks : ks + kl32],
                            rhs=wv_l,
                            start=True,
                            stop=not use_bias["bqkv"],
                        )
                        if use_bias["bqkv"]:
                            nc.tensor.matmul(
                                vps[:kl32, :],
                                lhsT=ones_row[:, :kl32],
                                rhs=bias_sb["bqkv_v"][0:1, l, :],
                                start=False,
                                stop=True,
                            )
                        nc.vector.tensor_copy(
                            out=va_tiles[(si, ki)][:kl, :, 0:16],
                            in_=vps[:kl, :].rearrange("k (h d) -> k h d", h=H),
                        )

                # attention q-tile stream with the layer tail (z/LN1/FFN/LN2)
                # emitted inline per 512-chunk as soon as its oT columns are
                # complete, so the tail overlaps the rest of attention.
                oT = tlp.tile([128, S + 32], bf16, tag="oT")
                h1 = sep.tile([128, NT, E], bf16, tag="h1")
                h1T = tlp.tile([128, NT, 128], bf16, tag="h1T")
                h = sep.tile([128, NT, E], bf16, tag="h")

                def tail_z(c):
                    zc = oap.tile([128, 512], f32, tag="oa", name=f"z_{li}_{c}")
                    stats = smallp.tile([128, 4, 6], f32, tag="stats", name="st1")
                    mv = smallp.tile([128, 4, 2], f32, tag="mv", name="mv1")
                    rstd = smallp.tile([128, 4], f32, tag="rstd", name="rstd1")
                    for tt in range(4):
                        t = 4 * c + tt
                        zt = zc[:, ts(tt, 128)]
                        nc.tensor.matmul(
                            zt, lhsT=hT[:, ts(t, 128)], rhs=ident, start=True, stop=False
                        )
                        nc.tensor.matmul(
                            zt,
                            lhsT=oT[:, ts(t, 128)],
                            rhs=woT_l,
                            start=False,
                            stop=not use_bias["bo"],
                        )
                        if use_bias["bo"]:
                            nc.tensor.matmul(
                                zt,
                                lhsT=ones_row,
                                rhs=bias_sb["bo_r"][0:1, l, :],
                                start=False,
                                stop=True,
                            )
                        nc.vector.bn_stats(out=stats[:, tt, :], in_=zt)
                        nc.vector.bn_aggr(out=mv[:, tt, :], in_=stats[:, tt, :])
                    nc.scalar.activation(
                        out=rstd, in_=mv[:, :, 1], func=AF.Sqrt, bias=eps_col
                    )
                    nc.vector.reciprocal(out=rstd, in_=rstd)
                    tpc = p512.tile([128, 512], f32, tag="mm512", name="tpc")
                    tpb = tpc.bitcast(bf16)
                    for tt in range(4):
                        t = 4 * c + tt
                        nc.vector.tensor_scalar(
                            out=h1[:, t, :],
                            in0=zc[:, ts(tt, 128)],
                            scalar1=mv[:, tt, 0:1],
                            scalar2=rstd[:, tt : tt + 1],
                            op0=OP.subtract,
                            op1=OP.mult,
                        )
                        if use_bias["ln"]:
                            nc.vector.tensor_mul(
                                out=h1[:, t, :], in0=h1[:, t, :],
                                in1=bias_sb["ln_gb"][:, l, 0, 0, :],
                            )
                            nc.vector.tensor_add(
                                out=h1[:, t, :], in0=h1[:, t, :],
                                in1=bias_sb["ln_gb"][:, l, 0, 1, :],
                            )
                        nc.tensor.transpose(
                            tpb[:, ts(tt, 128)], h1[:, t, :], ident
                        )
                    nc.vector.tensor_copy(
                        out=h1T[:, 4 * c : 4 * c + 4, :], in_=tpb[:, 0:512]
                    )

                def tail_ffn(c):
                    fTc = ffnp.tile([128, 4, 512], bf16, tag="fTc", name="fTc")
                    yc = oap.tile([128, 512], f32, tag="oa", name="yc")
                    stats2 = smallp.tile([128, 4, 6], f32, tag="stats", name="st2")
                    mv2 = smallp.tile([128, 4, 2], f32, tag="mv", name="mv2")
                    rstd2 = smallp.tile([128, 4], f32, tag="rstd", name="rstd2")
                    for tt in range(4):
                        t = 4 * c + tt
                        fps = p512.tile([128, 4, 128], f32, tag="mm512", name="fps")
                        for jt in range(4):
                            nc.tensor.matmul(
                                fps[:, jt, :],
                                lhsT=w1T_l[:, ts(jt, 128)],
                                rhs=h1T[:, t, :],
                                start=True,
                                stop=not use_bias["b1"],
                            )
                            if use_bias["b1"]:
                                nc.tensor.matmul(
                                    fps[:, jt, :],
                                    lhsT=ones_row,
                                    rhs=bias_sb["b1_c"][:, l, jt : jt + 1]
                                    .to_broadcast([1, 128]),
                                    start=False,
                                    stop=True,
                                )
                        if tt % 2:
                            nc.scalar.activation(
                                out=fTc[:, :, ts(tt, 128)], in_=fps, func=AF.Relu
                            )
                        else:
                            nc.vector.tensor_scalar(
                                out=fTc[:, :, ts(tt, 128)],
                                in0=fps,
                                scalar1=0.0,
                                scalar2=None,
                                op0=OP.max,
                            )
                        yt = yc[:, ts(tt, 128)]
                        nc.tensor.matmul(
                            yt, lhsT=h1T[:, t, :], rhs=ident, start=True, stop=False
                        )
                        for ft in range(4):
                            nc.tensor.matmul(
                                yt,
                                lhsT=fTc[:, ft, ts(tt, 128)],
                                rhs=w2T_l[:, ft, :],
                                start=False,
                                stop=(ft == 3) and not use_bias["b2"],
                            )
                        if use_bias["b2"]:
                            nc.tensor.matmul(
                                yt,
                                lhsT=ones_row,
                                rhs=bias_sb["b2_r"][0:1, l, :],
                                start=False,
                                stop=True,
                            )
                        nc.vector.bn_stats(out=stats2[:, tt, :], in_=yt)
                        nc.vector.bn_aggr(out=mv2[:, tt, :], in_=stats2[:, tt, :])
                    nc.scalar.activation(
                        out=rstd2, in_=mv2[:, :, 1], func=AF.Sqrt, bias=eps_col
                    )
                    nc.vector.reciprocal(out=rstd2, in_=rstd2)
                    if not is_last:
                        tpc2 = p512.tile([128, 512], f32, tag="mm512", name="tpc2")
                        tpb2 = tpc2.bitcast(bf16)
                    for tt in range(4):
                        t = 4 * c + tt
                        nc.vector.tensor_scalar(
                            out=h[:, t, :],
                            in0=yc[:, ts(tt, 128)],
                            scalar1=mv2[:, tt, 0:1],
                            scalar2=rstd2[:, tt : tt + 1],
                            op0=OP.subtract,
                            op1=OP.mult,
                        )
                        if use_bias["ln"]:
                            nc.vector.tensor_mul(
                                out=h[:, t, :], in0=h[:, t, :],
                                in1=bias_sb["ln_gb"][:, l, 1, 0, :],
                            )
                            nc.vector.tensor_add(
                                out=h[:, t, :], in0=h[:, t, :],
                                in1=bias_sb["ln_gb"][:, l, 1, 1, :],
                            )
                        if not is_last:
                            nc.tensor.transpose(
                                tpb2[:, ts(tt, 128)], h[:, t, :], ident
                            )
                    if not is_last:
                        hT_next = hT_tiles[(li + 1) % 2]
                        nc.vector.tensor_copy(
                            out=hT_next[:, ts(c, 512)], in_=tpb2[:, 0:512]
                        )
                        if (li + 1) not in qk_tiles:
                            alloc_qk_tiles(li + 1)
                        emit_qk_chunk(li + 1, layers[li + 1], hT_next, c)

                # grid tiles: all segment pieces within one 128-wide q tile
                # share st/pt (scores land at their q offsets in the free dim)
                # and one exp per k-depth, plus one oT copy per tile.
                grid = []
                for g in range(NT):
                    lo, hi = g * 128, (g + 1) * 128
                    pieces = []
                    for si, (ktiles, qtiles) in enumerate(seg_plans):
                        for qs, qn in qtiles:
                            if lo <= qs < hi:
                                pieces.append((si, ktiles, qs, qn))
                    grid.append(pieces)

                def emit_scores_tile(g):
                    pieces = grid[g]
                    mk = max(len(kt) for _, kt, _, _ in pieces)
                    pts = []
                    for ki in range(mk):
                        pk = [p for p in pieces if len(p[1]) > ki]
                        xlo = min(qs for _, _, qs, _ in pk) - 128 * g
                        xhi = max(qs + qn for _, _, qs, qn in pk) - 128 * g
                        st = stp.tile([128, H, 128], f32, tag="st", name="st")
                        for si, kt, qs, qn in pk:
                            ks, kl = kt[ki]
                            kl32 = _r32(kl)
                            qo = qs - 128 * g
                            for hh in range(H):
                                nc.tensor.matmul(
                                    st[:kl32, hh, qo : qo + qn],
                                    lhsT=kT2[:, hh, ks : ks + kl32],
                                    rhs=qT2[:, hh, qs : qs + qn],
                                    start=True,
                                    stop=True,
                                )
                        pt = ptp.tile([128, H, 128], bf16, tag="pt", name="pt")
                        nc.scalar.activation(
                            out=pt[:, :, xlo:xhi],
                            in_=st[:, :, xlo:xhi],
                            func=AF.Exp,
                            scale=SCALE,
                        )
                        pts.append(pt)
                    return pts

                def emit_av_tile(g, pts):
                    # pack up to 3 pieces' AV outputs into one PSUM slot
                    # (piece j at cols j*170) so one divide renormalizes all
                    pieces = grid[g]
                    out = []
                    for base in range(0, len(pieces), 3):
                        grp = pieces[base : base + 3]
                        om = oap.tile([128, 512], f32, tag="oa", name="oa")
                        qmax = 0
                        for j, (si, kt, qs, qn) in enumerate(grp):
                            nkt = len(kt)
                            qo = qs - 128 * g
                            qmax = max(qmax, qn)
                            oa = om[:, 170 * j : 170 * j + 136].rearrange(
                                "p (h x) -> p h x", h=H
                            )
                            for hh in range(H):
                                for ki in range(nkt):
                                    kl32 = _r32(kt[ki][1])
                                    nc.tensor.matmul(
                                        oa[:qn, hh, 0:17],
                                        lhsT=pts[ki][:kl32, hh, qo : qo + qn],
                                        rhs=va_tiles[(si, ki)][:kl32, hh, :],
                                        start=(ki == 0),
                                        stop=(ki == nkt - 1),
                                    )
                        oag = om[:, 0:510].rearrange(
                            "p (j x) -> p j x", j=3
                        )[:, 0 : len(grp), 0:136].rearrange(
                            "p j (h x) -> p j h x", h=H
                        )
                        rec = smallp.tile([128, 3, H], f32, tag="rec", name="rec")
                        nc.vector.reciprocal(
                            out=rec[:qmax, 0 : len(grp), :],
                            in_=oag[:qmax, :, :, 16],
                        )
                        oseg = osegp.tile([128, 3, E], f32, tag="oseg", name="oseg")
                        nc.vector.tensor_tensor(
                            out=oseg[:qmax, 0 : len(grp), :].rearrange(
                                "q j (h d) -> q j h d", h=H
                            ),
                            in0=oag[:qmax, :, :, 0:16],
                            in1=rec[:qmax, 0 : len(grp), :, None].to_broadcast(
                                [qmax, len(grp), H, 16]
                            ),
                            op=OP.mult,
                        )
                        out.append((om, oseg, grp))
                    return out

                def emit_transpose_tile(g, avs):
                    om0 = avs[0][0]
                    tp = om0[:, 144:272]
                    for om, oseg, grp in avs:
                        for j, (si, kt, qs, qn) in enumerate(grp):
                            qo = qs - 128 * g
                            nc.tensor.transpose(
                                tp[:, qo : qo + qn],
                                oseg[:qn, j, :],
                                ident_f[:qn, :qn],
                            )
                    nc.vector.tensor_copy(
                        out=oT[:, 128 * g : 128 * (g + 1)], in_=tp
                    )

                # 3-stage software pipeline per grid tile g:
                #   scores/exp(g+1) | transpose/copy(g-1) | AV+renorm(g)
                # plus chunk tails staggered in, so no engine waits in-order
                # on a cross-engine producer that hasn't been given slack.
                if os.environ.get("KDBG") == "noattn":
                    nc.vector.memset(oT[:, 0:S], 0.0)
                    for c in range(NC4):
                        tail_z(c)
                        tail_ffn(c)
                    continue
                sprev = None
                aprev = None
                todo = []
                for g in range(NT):
                    pts = emit_scores_tile(g)
                    if todo:
                        todo.pop(0)()
                    if aprev is not None:
                        emit_transpose_tile(aprev[0], aprev[1])
                        if aprev[0] % 4 == 3:
                            c = aprev[0] // 4
                            todo.append(lambda c=c: tail_z(c))
                            todo.append(lambda c=c: tail_ffn(c))
                    if sprev is not None:
                        avs = emit_av_tile(sprev[0], sprev[1])
                        aprev = (sprev[0], avs)
                    sprev = (g, pts)
                avs = emit_av_tile(sprev[0], sprev[1])
                emit_transpose_tile(aprev[0], aprev[1])
                emit_transpose_tile(sprev[0], avs)
                for f in todo:
                    f()
                tail_z(3)
                tail_ffn(3)

            # ---- mean pool over s ----
            om = oap.tile([128, H, 18], f32, tag="oa")
            acc = om[:, 0, 0:1]
            for t in range(NT):
                nc.tensor.matmul(
                    acc,
                    lhsT=h[:, t, :],
                    rhs=ones_col,
                    start=(t == 0),
                    stop=(t == NT - 1),
                )
            out_sb = smallp.tile([128, 1], f32, tag="out")
            nc.scalar.mul(out=out_sb, in_=acc, mul=1.0 / S)
            nc.sync.dma_start(out=out_d, in_=out_sb)

    split_multiwaits(nc)
    return nc


def _to_bf16(a):
    import ml_dtypes

    return np.asarray(a, np.float32).astype(ml_dtypes.bfloat16)


def _prep(x, boundaries, emb, Wqkv, bqkv, Wo, bo, W1, b1, W2, b2,
          ln1_g, ln1_b, ln2_g, ln2_b):
    x = np.asarray(x)
    emb = np.asarray(emb, np.float32)
    Wqkv = np.asarray(Wqkv, np.float32)
    Wo = np.asarray(Wo, np.float32)
    W1 = np.asarray(W1, np.float32)
    W2 = np.asarray(W2, np.float32)
    bqkv = np.asarray(bqkv, np.float32)
    bo = np.asarray(bo, np.float32)
    b1 = np.asarray(b1, np.float32)
    b2 = np.asarray(b2, np.float32)
    ln1_g = np.asarray(ln1_g, np.float32)
    ln1_b = np.asarray(ln1_b, np.float32)
    ln2_g = np.asarray(ln2_g, np.float32)
    ln2_b = np.asarray(ln2_b, np.float32)

    segs = _segments(boundaries)
    use_bias = {
        "bqkv": bool(np.any(bqkv != 0)),
        "bo": bool(np.any(bo != 0)),
        "b1": bool(np.any(b1 != 0)),
        "b2": bool(np.any(b2 != 0)),
        "ln": bool(
            np.any(ln1_g != 1) or np.any(ln1_b != 0)
            or np.any(ln2_g != 1) or np.any(ln2_b != 0)
        ),
    }

    # per-head q/k projection weights [e, L, qk, h, d]
    wqkh = (
        Wqkv[:, : 2 * E, :].reshape(L, 2, H, DH, E).transpose(4, 0, 1, 2, 3).copy()
    )

    shared = {
        "embt": _to_bf16(
            emb.reshape(2, 128, E).transpose(1, 0, 2).reshape(128, 2 * E)
        ),
        "iota": np.arange(V, dtype=np.float32).reshape(2, 128).T.copy(),
        "wvT": _to_bf16(Wqkv[:, 2 * E : 3 * E, :].transpose(2, 0, 1)),  # [e, L, E]
        "wqkh": _to_bf16(wqkh),
        "woT": _to_bf16(Wo.transpose(2, 0, 1)),  # [e, L, E]
        "w1T": _to_bf16(W1.transpose(2, 0, 1)),  # [e, L, F]
        "w2T": _to_bf16(
            W2.transpose(0, 2, 1).reshape(L, 4, 128, E).transpose(2, 0, 1, 3)
        ),
    }
    if use_bias["bqkv"]:
        shared["bqk_c"] = bqkv[:, : 2 * E].reshape(L, 2, 128).transpose(2, 0, 1).copy()
        shared["bqkv_v"] = bqkv[:, 2 * E : 3 * E].reshape(1, L, E).copy()
    if use_bias["bo"]:
        shared["bo_r"] = bo.reshape(1, L, E).copy()
    if use_bias["b1"]:
        shared["b1_c"] = b1.reshape(L, 4, 128).transpose(2, 0, 1).copy()
    if use_bias["b2"]:
        shared["b2_r"] = b2.reshape(1, L, E).copy()
    if use_bias["ln"]:
        ln_gb = np.stack(
            [np.stack([ln1_g, ln1_b], 1), np.stack([ln2_g, ln2_b], 1)], 1
        )  # [L, 2, 2, E]
        shared["ln_gb"] = ln_gb.reshape(1, L, 2, 2, E).copy()

    xf = [_to_bf16(x[b].reshape(1, S)) for b in range(B)]
    return segs, use_bias, shared, xf


def build_from_inputs(repeat=1, **inputs):
    segs, use_bias, shared, xf = _prep(**inputs)
    nc = bass.Bass()
    build(nc, segs, use_bias, repeat=repeat)
    in_maps = [dict(shared, xf=xf[b]) for b in range(B)]
    return nc, in_maps


def kernel(**inputs):
    from concourse.bass_utils import run_bass_kernel_spmd

    nc, in_maps = build_from_inputs(**inputs)
    res = run_bass_kernel_spmd(nc, in_maps, core_ids=list(range(B)))
    out = np.stack([res.results[b]["out"].reshape(E) for b in range(B)])
    return out.astype(np.float32)
